# revision 1
# baseline (speedup 1.0000x reference)
"""Multi-head causal attention on 8 Trainium2 NeuronCores.

Sharding: core c -> batch b = c // 4, head group g = c % 4 (4 of 16 heads).
Each core computes q/k/v for its 4 heads, causal softmax attention, and a
partial output  z_norm @ W_O[heads]  of shape [S, D].  Host sums the 4
head-group partials per batch and adds b_O.

Device kernel (per core, all matmuls in float32r = full-rate fp32):
  Phase A: qT/kT/vT[h] = W.T @ x.T   (+bias, q scaled by 1/sqrt(dh)),
           streamed to a DRAM scratch (SBUF cannot hold x.T + all outputs).
  Phase B: per head: scores = qT.T @ kT (i on partitions, j on free),
           additive causal mask on the diagonal 512-chunk, exp with fused
           row-sum accumulation on ACT, normalization folded into the PE
           transpose (rhs = identity * recip  instead of identity),
           zT += v_tile.T @ pT.
  Phase C: out[s_tile, d_chunk] = sum_h zT_h.T @ Wo_h.
"""

import sys

for _p in ("/opt/trn_rl_repo",):
    if _p not in sys.path:
        sys.path.insert(0, _p)

import numpy as np

import concourse.bass as bass
from concourse import bacc
import concourse.mybir as mybir
import concourse.tile as tile
from concourse.bass_utils import run_bass_kernel_spmd
from concourse.masks import make_identity

F32 = mybir.dt.float32
F32R = mybir.dt.float32r
BF16 = mybir.dt.float16  # fp16: same matmul rate, 4x finer mantissa than bf16

B, S, D, H, E = 2, 2048, 2048, 16, 128
HL = 4          # heads per core
NCORES = 8
P = 128         # partitions
CH = 512        # free-dim chunk
S_T = S // P    # 16 seq tiles
S_C = S // CH   # 4 seq chunks
D_T = D // P    # 16 model-dim subtiles
D_C = D // CH   # 4 model-dim chunks
INV_SQRT_E = 1.0 / float(np.sqrt(E))


def r(ap):
    return ap.bitcast(F32R)


def _trace_kernel(tc, xt, wq, wk, wv, wo, bq, bk, bv, outp):
    nc = tc.nc
    ts = bass.ts

    xt3 = xt.rearrange("(o p) s -> p o s", p=P)            # [128, 16, 2048]
    w3 = [w.rearrange("(o p) e -> p o e", p=P) for w in (wq, wk, wv)]
    wo3 = wo.rearrange("(h p) d -> p h d", p=P)            # [128, 4, 2048]
    out3 = outp.rearrange("(t p) d -> t p d", p=P)         # [16, 128, 2048]

    from contextlib import ExitStack

    with ExitStack() as top:
        const_pool = top.enter_context(tc.tile_pool(name="consts", bufs=1))
        dram = top.enter_context(tc.tile_pool(name="dram", bufs=1, space="DRAM"))

        # qT/kT/vT scratch: [12, 128, 2048] (m*4 + h)
        qkvT = dram.tile([3 * HL, P, S], F32R)

        identity_f = const_pool.tile([P, P], F32)
        make_identity(nc, identity_f)
        identity = const_pool.tile([P, P], F32R)
        nc.vector.tensor_copy(identity, identity_f)

        # additive causal masks for the diagonal 512-chunk, one per (i % 4)
        zpool = top.enter_context(tc.tile_pool(name="zT", bufs=1))
        zT = zpool.tile([P, HL, S], F32R)  # persists into phase C

        # transposed causal triangle for the diagonal 128-block of scoresT:
        # valid iff local col >= p  (j <= i)
        dmask = const_pool.tile([P, P], F32)
        nc.gpsimd.memset(dmask, 0.0)
        nc.gpsimd.affine_select(
            out=dmask,
            in_=dmask,
            compare_op=mybir.AluOpType.is_ge,
            fill=-1e9,
            base=0,
            pattern=[[1, P]],
            channel_multiplier=-1,
        )

        biases = const_pool.tile([P, 3, HL], F32)
        for m, bsrc in enumerate((bq, bk, bv)):
            nc.gpsimd.dma_start(biases[:, m, :], bsrc.rearrange("(h p) -> p h", p=P))

        # ---------------- Phase A: q/k/v projections ----------------
        with ExitStack() as pa:
            wpool = pa.enter_context(tc.tile_pool(name="wqkv", bufs=1))
            xpool = pa.enter_context(tc.tile_pool(name="xchunk", bufs=2))
            stage = pa.enter_context(tc.tile_pool(name="astage", bufs=3))
            psA = pa.enter_context(tc.tile_pool(name="psA", bufs=7, space="PSUM"))

            w_sb = [
                wpool.tile([P, D_T, HL * E], F32R, name=f"w{m}") for m in range(3)
            ]
            xc0 = xpool.tile([P, D_T, CH], F32R, name="xc")
            # interleave so matmul d=0 operands (xc d0, w0 d0) arrive first
            for d in range(D_T):
                nc.sync.dma_start(xc0[:, d, :], xt3[:, d, ts(0, CH)])
                for m in range(3):
                    nc.sync.dma_start(w_sb[m][:, d, :], w3[m][:, d, :])

            groups = [(m, h) for m in range(3) for h in range(HL)]
            for c in range(S_C):
                if c == 0:
                    xc = xc0
                else:
                    xc = xpool.tile([P, D_T, CH], F32R, name="xc")
                    for d in range(D_T):
                        nc.sync.dma_start(xc[:, d, :], xt3[:, d, ts(c, CH)])
                # d-outer in waves of 6 psum groups: PE consumes weight/x
                # slices at DMA-arrival order instead of stalling on full
                # weight tensors (matters for chunk 0)
                for wave in (groups[:6], groups[6:]):
                    pss = {g: psA.tile([P, CH], F32, name="psA") for g in wave}
                    for d in range(D_T):
                        for (m, h) in wave:
                            nc.tensor.matmul(
                                pss[(m, h)],
                                w_sb[m][:, d, ts(h, E)],
                                xc[:, d, :],
                                start=(d == 0),
                                stop=(d == D_T - 1),
                            )
                    for (m, h) in wave:
                        st = stage.tile([P, CH], F32R, name="st")
                        # q: bq is pre-scaled by 1/sqrt(E) on host, so
                        # (ps + b)/sqrt(E) == ps*scale + b_scaled
                        nc.vector.tensor_scalar(
                            st, pss[(m, h)],
                            INV_SQRT_E if m == 0 else 1.0,
                            biases[:, m, h, None],
                            op0=mybir.AluOpType.mult,
                            op1=mybir.AluOpType.add,
                        )
                        nc.sync.dma_start(qkvT[m * HL + h, :, ts(c, CH)], st)

        # ---------------- Phase B: attention per head ----------------
        # scoresT[j, i] computed directly (kT stationary, qT moving); exp only
        # over the valid column range; PV uses v augmented with a ones column
        # so z_ps[:, 128] holds softmax row-sums on the i partitions; z is
        # normalized per partition, transposed into zT for phase C.
        with ExitStack() as pb:
            qkvp = pb.enter_context(tc.tile_pool(name="qkv", bufs=2))
            vtp = pb.enter_context(tc.tile_pool(name="vtp", bufs=1))
            vnp = pb.enter_context(tc.tile_pool(name="vnat", bufs=2))
            epool = pb.enter_context(tc.tile_pool(name="expT", bufs=3))
            zsp = pb.enter_context(tc.tile_pool(name="zsb", bufs=4))
            small = pb.enter_context(tc.tile_pool(name="small", bufs=4))
            psS = pb.enter_context(tc.tile_pool(name="psS", bufs=4, space="PSUM"))
            psT = pb.enter_context(tc.tile_pool(name="psT", bufs=2, space="PSUM"))
            psZ = pb.enter_context(tc.tile_pool(name="psZ", bufs=2, space="PSUM"))

            for lh in range(HL):
                qT = qkvp.tile([P, S], F32R, name="qT")
                kT = qkvp.tile([P, S], F32R, name="kT")
                vT = vtp.tile([P, S], F32R, name="vT")
                for cc in range(S_C):
                    nc.sync.dma_start(qT[:, ts(cc, CH)], qkvT[0 * HL + lh, :, ts(cc, CH)])
                    nc.sync.dma_start(kT[:, ts(cc, CH)], qkvT[1 * HL + lh, :, ts(cc, CH)])
                    nc.sync.dma_start(vT[:, ts(cc, CH)], qkvT[2 * HL + lh, :, ts(cc, CH)])
                # v natural [j, e] + ones column (fused row-sums), fp16
                v_aug = vnp.tile([P, S_T, E + 1], BF16, name="v_aug")
                nc.vector.memset(v_aug[:, :, E : E + 1], 1.0)
                for jt in range(S_T):
                    tpv = psT.tile([P, P], F32R, name="tp")
                    nc.tensor.transpose(tpv, vT[:, ts(jt, P)], identity)
                    nc.vector.tensor_copy(v_aug[:, jt, :E], tpv)

                for c in range(S_C):     # i-chunks of 512
                    n_jt = S_C * c + 4
                    expT = epool.tile([P, S_T, CH], BF16, name="expT")
                    for jt in range(n_jt):
                        sps = psS.tile([P, CH], F32, name="sps")
                        nc.tensor.matmul(
                            sps,
                            kT[:, ts(jt, P)],
                            qT[:, ts(c, CH)],
                            start=True,
                            stop=True,
                        )
                        b = jt - S_C * c
                        if b >= 0:
                            # mask the 128-wide diagonal block; cols < b*128
                            # are never read by PV, cols beyond are valid
                            nc.vector.tensor_add(
                                sps[:, ts(b, P)], sps[:, ts(b, P)], dmask
                            )
                            nc.scalar.activation(
                                expT[:, jt, b * P :],
                                sps[:, b * P :],
                                mybir.ActivationFunctionType.Exp,
                            )
                        else:
                            nc.scalar.activation(
                                expT[:, jt, :],
                                sps,
                                mybir.ActivationFunctionType.Exp,
                            )
                    for a in range(S_C):  # i-tile within chunk
                        i = S_C * c + a
                        z_ps = psZ.tile([P, E + 1], F32, name="z_ps")
                        for jt in range(i + 1):
                            nc.tensor.matmul(
                                z_ps,
                                expT[:, jt, ts(a, P)],
                                v_aug[:, jt, :],
                                start=(jt == 0),
                                stop=(jt == i),
                            )
                        rec = small.tile([P, 1], F32, name="rec")
                        nc.vector.reciprocal(rec, z_ps[:, E : E + 1])
                        z_sb = zsp.tile([P, E], F32R, name="z_sb")
                        nc.vector.tensor_scalar_mul(z_sb, z_ps[:, :E], rec)
                        tpz = psT.tile([P, P], F32R, name="tp")
                        nc.tensor.transpose(tpz, z_sb, identity)
                        nc.vector.tensor_copy(zT[:, lh, ts(i, P)], tpz)

        # ---------------- Phase C: output projection ----------------
        with ExitStack() as pc:
            wop = pc.enter_context(tc.tile_pool(name="wo", bufs=1))
            ostage = pc.enter_context(tc.tile_pool(name="ostage", bufs=3))
            psC = pc.enter_context(tc.tile_pool(name="psC", bufs=2, space="PSUM"))

            wo_sb = wop.tile([P, HL, D], F32R)
            for lh in range(HL):
                for dc in range(D_C):
                    nc.sync.dma_start(wo_sb[:, lh, ts(dc, CH)], wo3[:, lh, ts(dc, CH)])

            for t in range(S_T):
                for dc in range(D_C):
                    ops = psC.tile([P, CH], F32, name="ops")
                    for lh in range(HL):
                        nc.tensor.matmul(
                            ops,
                            zT[:, lh, ts(t, P)],
                            wo_sb[:, lh, ts(dc, CH)],
                            start=(lh == 0),
                            stop=(lh == HL - 1),
                        )
                    ot = ostage.tile([P, CH], F32, name="ot")
                    nc.vector.tensor_copy(ot, ops)
                    nc.sync.dma_start(out3[t, :, ts(dc, CH)], ot)


_NC_CACHE = {}
LAST_RESULTS = None


def _get_nc():
    if "nc" not in _NC_CACHE:
        nc = bacc.Bacc("TRN2", target_bir_lowering=False, debug=False)
        xt = nc.dram_tensor("xt", [D, S], F32R, kind="ExternalInput")
        wq = nc.dram_tensor("wq", [D, HL * E], F32R, kind="ExternalInput")
        wk = nc.dram_tensor("wk", [D, HL * E], F32R, kind="ExternalInput")
        wv = nc.dram_tensor("wv", [D, HL * E], F32R, kind="ExternalInput")
        wo = nc.dram_tensor("wo", [HL * E, D], F32R, kind="ExternalInput")
        bq = nc.dram_tensor("bq", [HL * E], F32, kind="ExternalInput")
        bk = nc.dram_tensor("bk", [HL * E], F32, kind="ExternalInput")
        bv = nc.dram_tensor("bv", [HL * E], F32, kind="ExternalInput")
        outp = nc.dram_tensor("outp", [S, D], F32, kind="ExternalOutput")
        with tile.TileContext(nc) as tc:
            _trace_kernel(tc, xt, wq, wk, wv, wo, bq, bk, bv, outp)
        nc.compile()
        _NC_CACHE["nc"] = nc
    return _NC_CACHE["nc"]


def kernel(normalized_resid_pre, W_Q, W_K, W_V, W_O, b_Q, b_K, b_V, b_O):
    x = np.asarray(normalized_resid_pre, np.float32)
    W_Q = np.asarray(W_Q, np.float32)
    W_K = np.asarray(W_K, np.float32)
    W_V = np.asarray(W_V, np.float32)
    W_O = np.asarray(W_O, np.float32)
    b_Q = np.asarray(b_Q, np.float32)
    b_K = np.asarray(b_K, np.float32)
    b_V = np.asarray(b_V, np.float32)
    b_O = np.asarray(b_O, np.float32)

    nc = _get_nc()
    in_maps = []
    for core in range(NCORES):
        b, g = core // (NCORES // B), core % (NCORES // B)
        hs = range(g * HL, (g + 1) * HL)
        in_maps.append(
            {
                "xt": np.ascontiguousarray(x[b].T),
                "wq": np.ascontiguousarray(np.concatenate([W_Q[h] for h in hs], 1)),
                "wk": np.ascontiguousarray(np.concatenate([W_K[h] for h in hs], 1)),
                "wv": np.ascontiguousarray(np.concatenate([W_V[h] for h in hs], 1)),
                "wo": np.ascontiguousarray(
                    W_O[g * HL : (g + 1) * HL].reshape(HL * E, D)
                ),
                "bq": np.ascontiguousarray(b_Q[g * HL : (g + 1) * HL].reshape(-1) * np.float32(INV_SQRT_E)),
                "bk": np.ascontiguousarray(b_K[g * HL : (g + 1) * HL].reshape(-1)),
                "bv": np.ascontiguousarray(b_V[g * HL : (g + 1) * HL].reshape(-1)),
            }
        )

    res = run_bass_kernel_spmd(nc, in_maps, core_ids=list(range(NCORES)))
    global LAST_RESULTS
    LAST_RESULTS = res
    out = np.zeros((B, S, D), np.float32)
    for core in range(NCORES):
        out[core // (NCORES // B)] += res.results[core]["outp"]
    out += b_O[None, None, :]
    return out



# revision 2
# speedup vs baseline: 1.1144x; 1.1144x over previous
"""Multi-head causal attention on 8 Trainium2 NeuronCores — v2.

Sharding: core c -> batch b = c // 4, head group g = c % 4 (4 of 16 heads).
Each core computes q/k/v for its 4 heads, causal softmax attention, and a
partial output  z_norm @ W_O[heads]  of shape [S, D].  Host sums the 4
head-group partials per batch and adds b_O + sum_h b_V[h] @ W_O[h].

v2 design vs v1:
  - fp16 everywhere (x, W, q/k/v, exp, z, W_O); halves DMA, full-rate PE.
  - q/k/v SBUF-resident; no DRAM roundtrip between projection and attention.
  - v produced in natural [s, e] layout directly by phase A (x.T tile
    stationary) — kills the 64 v-transposes.  b_V folded into the host-side
    constant term (z = z~ + b_V exactly, since softmax rows sum to 1).
  - One global instruction stream scheduled by a greedy list-scheduler with
    per-op cost estimates: projection work of head h+1 (and the output
    projection at the tail) fills the PE while ACT runs head h's exp, which
    would otherwise bound the softmax phase.
"""

import sys

for _p in ("/opt/trn_rl_repo",):
    if _p not in sys.path:
        sys.path.insert(0, _p)

import numpy as np

import concourse.bass as bass
from concourse import bacc
import concourse.mybir as mybir
import concourse.tile as tile
from concourse.bass_utils import run_bass_kernel_spmd
from concourse.masks import make_identity

F32 = mybir.dt.float32
F16 = mybir.dt.float16

B, S, D, H, E = 2, 2048, 2048, 16, 128
HL = 4          # heads per core
NCORES = 8
P = 128         # partitions
CH = 512        # free-dim chunk
S_T = S // P    # 16 seq tiles
S_C = S // CH   # 4 seq chunks
D_T = D // P    # 16 model-dim subtiles
D_C = D // CH   # 4 model-dim chunks
INV_SQRT_E = 1.0 / float(np.sqrt(E))

ts = bass.ts


# ---------------------------------------------------------------------------
# Greedy list-scheduler.  Ops are closures tagged with an engine, a cost
# estimate, and dependencies.  Each engine executes its subsequence of the
# emission order in order, so emission order == execution order per engine.
# ---------------------------------------------------------------------------
class Sched:
    PE, ACT, DVE, GPS, DMA = "PE", "ACT", "DVE", "GPS", "DMA"

    def __init__(self):
        self.ops = {}          # id -> (engine, cost, deps, emit_fn, prio)
        self.order = []        # op ids, insertion order per priority class
        self.finish = {}
        self.emitted = set()

    def add(self, oid, engine, cost, deps, fn, prio=1):
        assert oid not in self.ops, oid
        self.ops[oid] = (engine, float(cost), list(deps), fn, prio)
        self.order.append(oid)
        return oid

    def run(self):
        cursor = {e: 0.0 for e in (self.PE, self.ACT, self.DVE, self.GPS, self.DMA)}
        remaining = list(self.order)
        emitted = self.emitted
        finish = self.finish

        def ready(oid):
            return all(d in emitted for d in self.ops[oid][2])

        def start_time(oid):
            eng, cost, deps, _, _ = self.ops[oid]
            t = cursor[eng]
            for d in deps:
                t = max(t, finish[d])
            return t

        def emit(oid):
            eng, cost, deps, fn, _ = self.ops[oid]
            t0 = start_time(oid)
            fn()
            finish[oid] = t0 + cost
            cursor[eng] = t0 + cost
            emitted.add(oid)
            remaining.remove(oid)

        while remaining:
            # 1. flush non-PE ops that are ready and would start at (or
            #    before) their engine cursor + small slack — keeps ACT/DVE/
            #    DMA queues fed without reordering them badly.
            progress = True
            while progress:
                progress = False
                for oid in list(remaining):
                    eng = self.ops[oid][0]
                    if eng == self.PE or not ready(oid):
                        continue
                    if start_time(oid) <= cursor[eng] + 2000.0:
                        emit(oid)
                        progress = True
            if not remaining:
                break
            # 2. pick a PE op: among ready PE ops prefer highest priority
            #    class that doesn't stall; else the op with minimum stall.
            pe_ready = [oid for oid in remaining if self.ops[oid][0] == self.PE and ready(oid)]
            if pe_ready:
                t_pe = cursor[self.PE]
                no_stall = [o for o in pe_ready if start_time(o) <= t_pe + 100.0]
                if no_stall:
                    # highest priority (lower number = more urgent), stable
                    best = min(no_stall, key=lambda o: self.ops[o][4])
                else:
                    best = min(pe_ready, key=start_time)
                emit(best)
                continue
            # 3. nothing PE-ready: force the earliest-startable remaining op
            rdy = [oid for oid in remaining if ready(oid)]
            if not rdy:
                raise RuntimeError(f"scheduler deadlock; remaining={remaining[:8]}")
            emit(min(rdy, key=start_time))
        return cursor


def _trace_kernel(tc, xt, wq, wk, wv, wo, bqk, outp):
    nc = tc.nc

    xt3 = xt.rearrange("(o p) s -> p o s", p=P)            # [128, 16, 2048]
    w3 = {
        "q": wq.rearrange("(o p) e -> p o e", p=P),        # [128, 16, 512]
        "k": wk.rearrange("(o p) e -> p o e", p=P),
        "v": wv.rearrange("(o p) e -> p o e", p=P),
    }
    wo3 = wo.rearrange("(h p) d -> p h d", p=P)            # [128, 4, 2048]
    out3 = outp.rearrange("(t p) d -> t p d", p=P)         # [16, 128, 2048]

    from contextlib import ExitStack

    sch = Sched()

    with ExitStack() as top:
        const_pool = top.enter_context(tc.tile_pool(name="consts", bufs=1))

        identity_f = const_pool.tile([P, P], F32)
        make_identity(nc, identity_f)
        identity = const_pool.tile([P, P], F16)
        nc.vector.tensor_copy(identity, identity_f)

        # additive causal mask for the diagonal 128-block of scoresT:
        # valid iff local col (i) >= partition (j)
        dmask = const_pool.tile([P, P], F32)
        nc.gpsimd.memset(dmask, 0.0)
        nc.gpsimd.affine_select(
            out=dmask,
            in_=dmask,
            compare_op=mybir.AluOpType.is_ge,
            fill=-1e9,
            base=0,
            pattern=[[1, P]],
            channel_multiplier=-1,
        )

        biases = const_pool.tile([P, 2, HL], F32)
        nc.gpsimd.dma_start(biases, bqk.rearrange("(m h p) -> p m h", p=P, m=2))

        # ---- persistent SBUF tensors -----------------------------------
        xpool = top.enter_context(tc.tile_pool(name="xsb", bufs=1))
        x_sb = xpool.tile([P, D_T, S], F16)                # 8.4 MB

        qkpool = top.enter_context(tc.tile_pool(name="qk", bufs=2))
        vpool = top.enter_context(tc.tile_pool(name="vaug", bufs=2))
        wpool = top.enter_context(tc.tile_pool(name="whead", bufs=2))
        zpool = top.enter_context(tc.tile_pool(name="zT", bufs=1))
        zT = zpool.tile([P, HL, S], F16)                   # 2.1 MB

        epool = top.enter_context(tc.tile_pool(name="expT", bufs=2))
        zsp = top.enter_context(tc.tile_pool(name="zsb", bufs=2))
        small = top.enter_context(tc.tile_pool(name="small", bufs=4))

        psQK = top.enter_context(tc.tile_pool(name="psQK", bufs=2, space="PSUM"))
        psV = top.enter_context(tc.tile_pool(name="psV", bufs=2, space="PSUM"))
        psS = top.enter_context(tc.tile_pool(name="psS", bufs=2, space="PSUM"))
        psZ = top.enter_context(tc.tile_pool(name="psZ", bufs=1, space="PSUM"))
        psT = top.enter_context(tc.tile_pool(name="psT", bufs=1, space="PSUM"))

        wopool = top.enter_context(tc.tile_pool(name="wo", bufs=1))
        wo_sb = wopool.tile([P, HL, D], F16)               # 2.1 MB
        opool = top.enter_context(tc.tile_pool(name="ostage", bufs=4))

        # =================================================================
        # Build the op graph.
        # =================================================================
        DMA_NS = lambda nbytes: max(700.0, nbytes / 300.0)  # ns, ~300B/ns

        # ---- prologue DMAs (x, head-0 weights, biases, wo) --------------
        # x chunk DMAs per (c, d) so phase-A matmuls can chase arrival.
        xdma = {}
        wdma = {}

        def w_dma(h):
            w_sb = wpool.tile([P, D_T, 3 * E], F16, name="whead")
            for m_i, m in enumerate(("q", "k", "v")):
                def fn(m=m, m_i=m_i, h=h, w_sb=w_sb):
                    nc.sync.dma_start(
                        w_sb[:, :, ts(m_i, E)], w3[m][:, :, ts(h, E)]
                    )
                wdma[(h, m)] = sch.add(
                    f"wdma_{h}_{m}", Sched.DMA, DMA_NS(P * D_T * E * 2), [], fn,
                    prio=0,
                )
            return w_sb

        w_sb_h = {}
        w_sb_h[0] = w_dma(0)

        for c in range(S_C):
            for d in range(D_T):
                def fn(c=c, d=d):
                    nc.sync.dma_start(x_sb[:, d, ts(c, CH)], xt3[:, d, ts(c, CH)])
                xdma[(c, d)] = sch.add(
                    f"xdma_{c}_{d}", Sched.DMA, DMA_NS(P * CH * 2), [], fn, prio=0
                )

        def wo_dma():
            ids = []
            for dc in range(D_C):
                def fn(dc=dc):
                    nc.sync.dma_start(
                        wo_sb[:, :, ts(dc, CH)], wo3[:, :, ts(dc, CH)]
                    )
                ids.append(
                    sch.add(f"wodma_{dc}", Sched.DMA, DMA_NS(P * HL * CH * 2), [], fn,
                            prio=0)
                )
            return ids

        # ---- phase A ops for one head -----------------------------------
        # qk: 8 psum groups (m in {q,k} x 4 chunks), 16 d-matmuls each.
        # v:  16 psum groups (s-tile), 16 d-matmuls each, natural layout.
        qk_sb_h = {}
        vaug_h = {}
        qk_evac = {}   # (h, m_i, c) -> op id
        v_evac = {}    # (h, t) -> op id
        qk_seq = []    # all qk evac ids in emission order (psQK slot deps)
        v_seq = []

        def build_A(h):
            qk_sb = qkpool.tile([P, 2, S], F16, name="qk")
            qk_sb_h[h] = qk_sb
            v_aug = vpool.tile([P, S_T, E + 1], F16, name="vaug")
            vaug_h[h] = v_aug
            w_sb = w_sb_h[h]

            def memset_ones(v_aug=v_aug):
                nc.vector.memset(v_aug[:, :, E : E + 1], 1.0)
            sch.add(f"vones_{h}", Sched.DVE, 300, [], memset_ones, prio=2)

            for m_i, m in enumerate(("q", "k")):
                for c in range(S_C):
                    ps = psQK.tile([P, CH], F32, name="psQK")

                    def mm(ps=ps, m_i=m_i, c=c, w_sb=w_sb):
                        for d in range(D_T):
                            nc.tensor.matmul(
                                ps,
                                w_sb[:, d, ts(m_i, E)],
                                x_sb[:, d, ts(c, CH)],
                                start=(d == 0),
                                stop=(d == D_T - 1),
                            )

                    deps = [wdma[(h, m)]] + [xdma[(c, d)] for d in range(D_T)]
                    # psQK slot: wait for evac 2 groups back
                    if len(qk_seq) >= 2:
                        deps.append(qk_seq[-2])
                    mm_id = sch.add(
                        f"qkmm_{h}_{m}_{c}", Sched.PE, D_T * CH / 2.4 + 120,
                        deps, mm, prio=3,
                    )

                    def evac(ps=ps, m_i=m_i, c=c, h=h, qk_sb=qk_sb):
                        nc.vector.tensor_scalar(
                            qk_sb[:, m_i, ts(c, CH)], ps,
                            INV_SQRT_E if m_i == 0 else 1.0,
                            biases[:, m_i, h, None],
                            op0=mybir.AluOpType.mult,
                            op1=mybir.AluOpType.add,
                        )
                    ev = sch.add(
                        f"qkev_{h}_{m}_{c}", Sched.DVE, 550, [mm_id], evac, prio=3
                    )
                    qk_evac[(h, m_i, c)] = ev
                    qk_seq.append(ev)

            for t in range(S_T):
                ps = psV.tile([P, E], F32, name="psV")

                def mmv(ps=ps, t=t, w_sb=w_sb):
                    for d in range(D_T):
                        nc.tensor.matmul(
                            ps,
                            x_sb[:, d, ts(t, P)],
                            w_sb[:, d, ts(2, E)],
                            start=(d == 0),
                            stop=(d == D_T - 1),
                        )

                deps = [wdma[(h, "v")]] + [xdma[(t // 4, d)] for d in range(D_T)]
                if len(v_seq) >= 2:
                    deps.append(v_seq[-2])
                mm_id = sch.add(
                    f"vmm_{h}_{t}", Sched.PE, D_T * E / 2.4 + 16 * 60 + 120,
                    deps, mmv, prio=4,
                )

                def evacv(ps=ps, t=t, v_aug=v_aug):
                    nc.vector.tensor_copy(v_aug[:, t, :E], ps)
                ev = sch.add(f"vev_{h}_{t}", Sched.DVE, 380, [mm_id], evacv, prio=4)
                v_evac[(h, t)] = ev
                v_seq.append(ev)

        # ---- phase B ops for one head -----------------------------------
        exp_ops = {}    # (h, c, jt) -> ACT op id
        s_seq = []      # score matmul ids in order (psS slot deps)
        exp_seq = []
        pv_mul = {}     # (h, c, a) -> DVE mul op id (frees psZ)
        tz_copy = {}    # (h, t) -> DVE zT copy id (frees psT; gates phase C)
        pv_seq = []
        tz_seq = []

        def build_B(h):
            qk_sb = qk_sb_h[h]
            v_aug = vaug_h[h]
            expT = {}
            for c in range(S_C):
                expT[c] = epool.tile([P, S_T, CH], F16, name="expT")
                n_jt = S_C * c + 4
                for jt in range(n_jt):
                    b = jt - S_C * c
                    lo = max(b, 0) * P          # first valid i-col in chunk
                    w_i = CH - lo
                    ps = psS.tile([P, CH], F32, name="psS")

                    def mm(ps=ps, c=c, jt=jt, lo=lo, qk_sb=qk_sb):
                        nc.tensor.matmul(
                            ps[:, lo:],
                            qk_sb[:, 1, ts(jt, P)],
                            qk_sb[:, 0, c * CH + lo : (c + 1) * CH],
                            start=True,
                            stop=True,
                        )

                    deps = [qk_evac[(h, 0, c)], qk_evac[(h, 1, jt // 4)]]
                    if len(s_seq) >= 2:
                        deps.append(exp_seq[-2])
                    mm_id = sch.add(
                        f"smm_{h}_{c}_{jt}", Sched.PE, w_i / 2.4 + 90, deps, mm,
                        prio=2,
                    )
                    s_seq.append(mm_id)

                    prev = mm_id
                    if b >= 0:
                        def mask(ps=ps, b=b):
                            nc.vector.tensor_add(ps[:, ts(b, P)], ps[:, ts(b, P)], dmask)
                        prev = sch.add(
                            f"mask_{h}_{c}_{jt}", Sched.DVE, 420, [mm_id], mask,
                            prio=2,
                        )

                    def ex(ps=ps, c=c, jt=jt, lo=lo, expT_c=expT[c]):
                        nc.scalar.activation(
                            expT_c[:, jt, lo:],
                            ps[:, lo:],
                            mybir.ActivationFunctionType.Exp,
                        )
                    eid = sch.add(
                        f"exp_{h}_{c}_{jt}", Sched.ACT, 230 + w_i * 0.85, [prev],
                        ex, prio=2,
                    )
                    exp_ops[(h, c, jt)] = eid
                    exp_seq.append(eid)

                for a in range(S_C):
                    i = S_C * c + a
                    z_ps = psZ.tile([P, E + 1], F32, name="psZ")

                    def mmpv(z_ps=z_ps, c=c, a=a, i=i, v_aug=v_aug, expT_c=expT[c]):
                        for jt in range(i + 1):
                            nc.tensor.matmul(
                                z_ps,
                                expT_c[:, jt, ts(a, P)],
                                v_aug[:, jt, :],
                                start=(jt == 0),
                                stop=(jt == i),
                            )

                    deps = [exp_ops[(h, c, jt)] for jt in range(i + 1)]
                    deps += [v_evac[(h, jt)] for jt in range(i + 1)]
                    deps.append(f"vones_{h}")
                    if pv_seq:
                        deps.append(pv_seq[-1])     # psZ bufs=1
                    pv_id = sch.add(
                        f"pv_{h}_{c}_{a}", Sched.PE, (i + 1) * (129 / 2.4 + 28) + 90,
                        deps, mmpv, prio=2,
                    )

                    z_sb = zsp.tile([P, E], F16, name="zsb")
                    rec = small.tile([P, 1], F32, name="rec")

                    def norm(z_ps=z_ps, z_sb=z_sb, rec=rec):
                        nc.vector.reciprocal(rec, z_ps[:, E : E + 1])
                        nc.vector.tensor_scalar_mul(z_sb, z_ps[:, :E], rec)
                    mul_id = sch.add(
                        f"pvmul_{h}_{c}_{a}", Sched.DVE, 550, [pv_id], norm, prio=2
                    )
                    pv_mul[(h, c, a)] = mul_id
                    pv_seq.append(mul_id)

                    tp = psT.tile([P, P], F16, name="tp")

                    def tz(tp=tp, z_sb=z_sb):
                        nc.tensor.transpose(tp, z_sb, identity)
                    tz_deps = [mul_id]
                    if tz_seq:
                        tz_deps.append(tz_seq[-1])  # psT bufs=1
                    tz_id = sch.add(
                        f"tz_{h}_{c}_{a}", Sched.PE, 140, tz_deps, tz, prio=2
                    )

                    def tzc(tp=tp, i=i, h=h):
                        nc.vector.tensor_copy(zT[:, h, ts(i, P)], tp)
                    tc_id = sch.add(
                        f"tzc_{h}_{c}_{a}", Sched.DVE, 380, [tz_id], tzc, prio=2
                    )
                    tz_copy[(h, i)] = tc_id
                    tz_seq.append(tc_id)

        # ---- phase C ops -------------------------------------------------
        c_seq = []

        def build_C(wo_ids):
            for t in range(S_T):
                for dc in range(D_C):
                    ps = psQK.tile([P, CH], F32, name="psQK")

                    def mm(ps=ps, t=t, dc=dc):
                        for lh in range(HL):
                            nc.tensor.matmul(
                                ps,
                                zT[:, lh, ts(t, P)],
                                wo_sb[:, lh, ts(dc, CH)],
                                start=(lh == 0),
                                stop=(lh == HL - 1),
                            )

                    deps = [tz_copy[(lh, t)] for lh in range(HL)] + wo_ids
                    if len(qk_seq) + len(c_seq) >= 2:
                        allq = qk_seq + c_seq
                        deps.append(allq[-2])
                    mm_id = sch.add(
                        f"cmm_{t}_{dc}", Sched.PE, HL * (CH / 2.4 + 28) + 90,
                        deps, mm, prio=5,
                    )

                    ot = opool.tile([P, CH], F32, name="ot")

                    def evac(ps=ps, ot=ot):
                        nc.vector.tensor_copy(ot, ps)
                    ev = sch.add(f"cev_{t}_{dc}", Sched.DVE, 780, [mm_id], evac,
                                 prio=5)
                    c_seq.append(ev)

                    def dma(ot=ot, t=t, dc=dc):
                        nc.sync.dma_start(out3[t, :, ts(dc, CH)], ot)
                    sch.add(f"cdma_{t}_{dc}", Sched.DMA, DMA_NS(P * CH * 4),
                            [ev], dma, prio=5)

        # ---- assemble ---------------------------------------------------
        build_A(0)
        for h in range(HL):
            if h + 1 < HL:
                w_sb_h[h + 1] = w_dma(h + 1)
                build_A(h + 1)
                build_B(h)
            else:
                wo_ids = wo_dma()
                build_B(h)
                build_C(wo_ids)

        sch.run()


_NC_CACHE = {}
LAST_RESULTS = None


def _get_nc():
    if "nc" not in _NC_CACHE:
        nc = bacc.Bacc("TRN2", target_bir_lowering=False, debug=False)
        xt = nc.dram_tensor("xt", [D, S], F16, kind="ExternalInput")
        wq = nc.dram_tensor("wq", [D, HL * E], F16, kind="ExternalInput")
        wk = nc.dram_tensor("wk", [D, HL * E], F16, kind="ExternalInput")
        wv = nc.dram_tensor("wv", [D, HL * E], F16, kind="ExternalInput")
        wo = nc.dram_tensor("wo", [HL * E, D], F16, kind="ExternalInput")
        bqk = nc.dram_tensor("bqk", [2 * HL * E], F32, kind="ExternalInput")
        outp = nc.dram_tensor("outp", [S, D], F32, kind="ExternalOutput")
        with tile.TileContext(nc) as tc:
            _trace_kernel(tc, xt, wq, wk, wv, wo, bqk, outp)
        nc.compile()
        _NC_CACHE["nc"] = nc
    return _NC_CACHE["nc"]


def kernel(normalized_resid_pre, W_Q, W_K, W_V, W_O, b_Q, b_K, b_V, b_O):
    x = np.asarray(normalized_resid_pre, np.float32)
    W_Q = np.asarray(W_Q, np.float32)
    W_K = np.asarray(W_K, np.float32)
    W_V = np.asarray(W_V, np.float32)
    W_O = np.asarray(W_O, np.float32)
    b_Q = np.asarray(b_Q, np.float32)
    b_K = np.asarray(b_K, np.float32)
    b_V = np.asarray(b_V, np.float32)
    b_O = np.asarray(b_O, np.float32)

    nc = _get_nc()
    in_maps = []
    for core in range(NCORES):
        b, g = core // (NCORES // B), core % (NCORES // B)
        hs = list(range(g * HL, (g + 1) * HL))
        bqk = np.concatenate(
            [b_Q[hs].reshape(-1) * np.float32(INV_SQRT_E), b_K[hs].reshape(-1)]
        )
        in_maps.append(
            {
                "xt": np.ascontiguousarray(x[b].T).astype(np.float16),
                "wq": np.ascontiguousarray(
                    np.concatenate([W_Q[h] for h in hs], 1)
                ).astype(np.float16),
                "wk": np.ascontiguousarray(
                    np.concatenate([W_K[h] for h in hs], 1)
                ).astype(np.float16),
                "wv": np.ascontiguousarray(
                    np.concatenate([W_V[h] for h in hs], 1)
                ).astype(np.float16),
                "wo": np.ascontiguousarray(
                    W_O[hs].reshape(HL * E, D)
                ).astype(np.float16),
                "bqk": np.ascontiguousarray(bqk),
            }
        )

    res = run_bass_kernel_spmd(nc, in_maps, core_ids=list(range(NCORES)))
    global LAST_RESULTS
    LAST_RESULTS = res
    out = np.zeros((B, S, D), np.float32)
    for core in range(NCORES):
        out[core // (NCORES // B)] += res.results[core]["outp"]
    # bias terms handled on host: b_O plus the b_V contribution, which is
    # exact because softmax rows sum to 1:  z = z~ + b_V.
    const = b_O + np.einsum("he,hed->d", b_V, W_O)
    out += const[None, None, :]
    return out
